# revision 19
# baseline (speedup 1.0000x reference)
"""Trainium2 Bass kernel for nn_Attention_3126736192307 — merged pipeline.

Causal multi-head attention with RoPE: B=2, S=2048, H=2048, 16 heads x 128.

Sharding (tensor parallel over heads, 8 cores, 2 heads each):
  - Wq/Wk/Wv column-split (per-head), Wo row-split; each core computes a
    partial [B*S, H] output; the host sums the 8 partials.

v3: merged pipeline at tchunk=512. Chunk-step j emits
  proj(j) -> attn(j-1) -> rope(j)
so attention for chunk j-1 follows chunk j's projections in the PE FIFO
while j's RoPE (DVE) and PSUM evictions (ScalarE) run in their shadows.

PSUM discipline: a matmul accumulation group zeroes its whole 2KB bank at
start, so every concurrently-open group owns a bank. Projections run as
SEQUENTIAL groups (q_h0, q_h1, k_h0, k_h1, v0..v3) rotating 2 banks with
X.T chunk-resident in SBUF; attention processes one head at a time so a
single colsum + a single AV group are open at once (2+1 banks), and
score tiles rotate 3 banks shared with the Wo output tiles. 2+3+2+1 = 8.

Per-core dataflow (all matmuls transpose-free by construction):
  - Host pre-transposes: X.T [H, T], WqT/WkT [H, 256] (head-dim permuted so
    RoPE's rotate_half becomes an intra-quadrant stream_shuffle), WvT [H, 256],
    WoT [256, H], cos/sin [128, T] feature-major bf16 (sin sign-folded).
  - scores.T [j,i] = k.T (lhsT) @ q.T; exp on ScalarE (no max subtraction:
    scores ~N(0,1) after the 1/sqrt(hd) scale); causal block skipping + 0/1
    mask multiply on diagonal-crossing tiles; column sums via ones-matmul on
    TensorE; AV accumulation in PSUM; normalization folded into eviction.
  - out.T (lhsT) @ WoT -> partial [T, H]; PSUM->SBUF evictions alternate
    DVE/ScalarE and the DRAM stores alternate two DMA queues.
"""

import os
import sys

for _p in ("/opt/trn_rl_repo", "/root/.axon_site/_ro/trn_rl_repo"):
    if os.path.isdir(_p) and _p not in sys.path:
        sys.path.append(_p)

from contextlib import ExitStack

import ml_dtypes
import numpy as np

import concourse.bass as bass
import concourse.bacc as bacc
import concourse.tile as tile
from concourse import mybir
from concourse.bass_utils import run_bass_kernel_spmd

B, S, H, NH = 2, 2048, 2048, 16
HD = 128
NCORES = 8
HPC = NH // NCORES            # heads per core = 2
M = HPC * HD                  # 256 output channels per core
SCALE = HD ** -0.5
P = 128                       # partitions
NKT = H // P                  # 16 contraction tiles for projections

F32 = mybir.dt.float32

# head-dim permutation: interleave halves at 16 granularity so the RoPE
# partner (d <-> d+64) sits 16 partitions away inside one 32-part quadrant
PERM = np.concatenate([np.arange(16 * m, 16 * m + 16) + (64 if odd else 0)
                       for m in range(4) for odd in (0, 1)])
SWAP_MASK = [i ^ 16 for i in range(32)]

BF16 = ml_dtypes.bfloat16

TCHUNK = 512


def build_masks(tchunk):
    """0/1 keep-masks for the R diagonal-crossing j-tiles of each i-chunk."""
    r = tchunk // P
    m = np.zeros((r, P, tchunk), np.float32)
    il = np.arange(tchunk)
    for ri in range(r):
        for jl in range(P):
            m[ri, jl, :] = (P * ri + jl <= il).astype(np.float32)
    return m


def build_nc(s=S, b=B, tchunk=TCHUNK, mm_dtype=mybir.dt.bfloat16):
    t = b * s
    nch = t // tchunk             # 8 chunk-steps
    ich = s // tchunk             # 4 attention i-chunks per batch
    r_mask = tchunk // P          # 4 diagonal-crossing tiles per i-chunk
    ntt = t // P                  # 32 token tiles
    nvp = tchunk // P             # 4 v sub-tiles per chunk
    spt = s // P                  # 16 j-tiles per batch

    FR = mm_dtype
    EXP = mybir.ActivationFunctionType.Exp

    nc = bacc.Bacc("TRN2", target_bir_lowering=False, debug=False)

    xt = nc.declare_dram_parameter("xt", [H, t], FR, isOutput=False)
    wqt = nc.declare_dram_parameter("wqt", [H, M], FR, isOutput=False)
    wkt = nc.declare_dram_parameter("wkt", [H, M], FR, isOutput=False)
    wvt = nc.declare_dram_parameter("wvt", [H, M], FR, isOutput=False)
    wot = nc.declare_dram_parameter("wot", [M, H], FR, isOutput=False)
    cost = nc.declare_dram_parameter("cost", [HD, t], FR, isOutput=False)
    sint = nc.declare_dram_parameter("sint", [HD, t], FR, isOutput=False)
    masks = nc.declare_dram_parameter("masks", [r_mask, P, tchunk], FR,
                                      isOutput=False)
    out = nc.declare_dram_parameter("out", [t, H], FR, isOutput=True)

    with tile.TileContext(nc) as tc, ExitStack() as ctx:
        persist = ctx.enter_context(tc.tile_pool(name="persist", bufs=1))

        qr = [persist.tile([P, t], FR, tag=f"qr{h}", name=f"qr{h}")
              for h in range(HPC)]
        kr = [persist.tile([P, t], FR, tag=f"kr{h}", name=f"kr{h}")
              for h in range(HPC)]
        vv = persist.tile([P, ntt, M], FR, tag="vv")   # v[tt*128+p, d]
        ones_s = persist.tile([P, P], FR, tag="ones")
        nc.vector.memset(ones_s[:], 1.0)
        mask_s = persist.tile([P, r_mask, tchunk], FR, tag="masks")
        wo_s = persist.tile([P, HPC, H], FR, tag="wo")
        cos_sb = persist.tile([P, t], FR, tag="cosb")
        sin_sb = persist.tile([P, t], FR, tag="sinb")
        wq_s = persist.tile([P, NKT, M], FR, tag="wq")
        wk_s = persist.tile([P, NKT, M], FR, tag="wk")
        wv_s = persist.tile([P, NKT, M], FR, tag="wv")

        xt_pool = ctx.enter_context(tc.tile_pool(name="xtp", bufs=2))
        qkt_pool = ctx.enter_context(tc.tile_pool(name="qkt", bufs=2))
        rtmp_pool = ctx.enter_context(tc.tile_pool(name="rtmp", bufs=3))
        exps_pool = ctx.enter_context(tc.tile_pool(name="exps", bufs=6))
        rcp_pool = ctx.enter_context(tc.tile_pool(name="rcp", bufs=2))
        outT_pool = ctx.enter_context(tc.tile_pool(name="outT", bufs=2))
        ev_pool = ctx.enter_context(tc.tile_pool(name="evp", bufs=8))
        # PSUM: 8 banks: PR0 PR1 (sequential projection groups), S0 S1 S2
        # (score j-tiles + Wo outputs), C0 C1 (per-head colsums), AV
        pPR = ctx.enter_context(tc.tile_pool(name="pPR", bufs=1, space="PSUM"))
        pS = ctx.enter_context(tc.tile_pool(name="pS", bufs=1, space="PSUM"))
        pC = ctx.enter_context(tc.tile_pool(name="pC", bufs=1, space="PSUM"))
        pAV = ctx.enter_context(tc.tile_pool(name="pAV", bufs=1, space="PSUM"))

        # upfront loads on the gpsimd SWDGE queue in order of first use;
        # kt0 slivers first so the opening matmuls aren't gated on bulk DMAs
        for w_s, wsrc in ((wq_s, wqt), (wk_s, wkt), (wv_s, wvt)):
            nc.gpsimd.dma_start(
                out=w_s[:, 0:1, :],
                in_=wsrc[0:P, :].rearrange("(k p) m -> p k m", p=P))
        KWG = 5                     # weight k-tiles per bulk DMA (1+5+5+5)
        for lo_kt in range(1, NKT, KWG):
            hi_kt = min(lo_kt + KWG, NKT)
            gsl = slice(lo_kt * P, hi_kt * P)
            for w_s, wsrc in ((wq_s, wqt), (wk_s, wkt), (wv_s, wvt)):
                nc.gpsimd.dma_start(
                    out=w_s[:, lo_kt:hi_kt, :],
                    in_=wsrc[gsl, :].rearrange("(k p) m -> p k m", p=P))
        nc.gpsimd.dma_start(out=cos_sb[:], in_=cost[:, :])
        nc.gpsimd.dma_start(out=sin_sb[:], in_=sint[:, :])
        nc.gpsimd.dma_start(out=mask_s[:],
                            in_=masks.rearrange("r p n -> p r n"))
        nc.gpsimd.dma_start(out=wo_s[:],
                            in_=wot.rearrange("(mt p) o -> p mt o", p=P))

        pr_idx = [0]                  # rotating projection bank
        s_idx = [0]                   # rotating score/wo bank

        def emit_proj0():
            """Chunk 0: kt-interleaved groups across all 8 banks so the PE
            consumes each xt k-tile as it lands (startup is DMA-bound)."""
            j = 0
            tsl = slice(0, tchunk)
            xt_s = xt_pool.tile([P, NKT, tchunk], FR, tag="xt", name="xt0")
            nc.sync.dma_start(
                out=xt_s[:, 0:1, :],
                in_=xt[0:P, tsl].rearrange("(k p) t -> p k t", p=P))
            nc.sync.dma_start(
                out=xt_s[:, 1:4, :],
                in_=xt[P:4 * P, tsl].rearrange("(k p) t -> p k t", p=P))
            for g in range(1, 4):
                nc.sync.dma_start(
                    out=xt_s[:, 4 * g:4 * g + 4, :],
                    in_=xt[4 * g * P:(4 * g + 4) * P, tsl].rearrange(
                        "(k p) t -> p k t", p=P))
            qk_ps = [pPR.tile([P, tchunk], F32, tag="PR0", name="q0_0"),
                     pPR.tile([P, tchunk], F32, tag="PR1", name="q0_1"),
                     pS.tile([P, tchunk], F32, tag="S0", name="k0_0"),
                     pS.tile([P, tchunk], F32, tag="S1", name="k0_1")]
            v_ps = [pS.tile([P, tchunk], F32, tag="S2", name="v0_0"),
                    pC.tile([P, tchunk], F32, tag="C0", name="v0_1"),
                    pC.tile([P, tchunk], F32, tag="C1", name="v0_2"),
                    pAV.tile([P, tchunk], F32, tag="AV", name="v0_3")]
            for kt in range(NKT):
                fl = dict(start=(kt == 0), stop=(kt == NKT - 1))
                for wi, w_s in ((0, wq_s), (1, wk_s)):
                    for h in range(HPC):
                        msl = slice(h * P, (h + 1) * P)
                        nc.tensor.matmul(qk_ps[wi * HPC + h][:],
                                         w_s[:, kt, msl], xt_s[:, kt, :],
                                         **fl)
                for ts_ in range(nvp):
                    ssl = slice(ts_ * P, (ts_ + 1) * P)
                    nc.tensor.matmul(v_ps[ts_][:, :M], xt_s[:, kt, ssl],
                                     wv_s[:, kt, :], **fl)
            qkt = qkt_pool.tile([P, 2, HPC, tchunk], FR, tag="qkt",
                                name="qkt0")
            for wi in range(2):
                for h in range(HPC):
                    nc.scalar.copy(out=qkt[:, wi, h, :],
                                   in_=qk_ps[wi * HPC + h][:])
            for ts_ in range(nvp):
                nc.vector.tensor_copy(out=vv[:, ts_, :],
                                      in_=v_ps[ts_][:, :M])
            return qkt

        def emit_proj(j):
            tsl = slice(j * tchunk, (j + 1) * tchunk)
            # chunk-resident X.T: [P, NKT, tchunk] bf16 (16KB/partition)
            xt_s = xt_pool.tile([P, NKT, tchunk], FR, tag="xt",
                                name=f"xt{j}")
            for lo_kt in range(0, NKT, 8):
                nc.sync.dma_start(
                    out=xt_s[:, lo_kt:lo_kt + 8, :],
                    in_=xt[lo_kt * P:(lo_kt + 8) * P, tsl].rearrange(
                        "(k p) t -> p k t", p=P))
            qkt = qkt_pool.tile([P, 2, HPC, tchunk], FR, tag="qkt",
                                name=f"qkt{j}")
            # sequential accumulation groups, 2-bank rotation; evictions
            # (ScalarE q/k, DVE v) chase the groups
            for wi, w_s in ((0, wq_s), (1, wk_s)):
                for h in range(HPC):
                    ps = pPR.tile([P, tchunk], F32,
                                  tag=f"PR{pr_idx[0] % 2}",
                                  name=f"p{j}_{wi}_{h}")
                    pr_idx[0] += 1
                    msl = slice(h * P, (h + 1) * P)
                    for kt in range(NKT):
                        nc.tensor.matmul(ps[:], w_s[:, kt, msl],
                                         xt_s[:, kt, :],
                                         start=(kt == 0),
                                         stop=(kt == NKT - 1))
                    nc.scalar.copy(out=qkt[:, wi, h, :], in_=ps[:])
            for ts_ in range(nvp):
                ps = pPR.tile([P, tchunk], F32, tag=f"PR{pr_idx[0] % 2}",
                              name=f"pv{j}_{ts_}")
                pr_idx[0] += 1
                ssl = slice(ts_ * P, (ts_ + 1) * P)
                for kt in range(NKT):
                    nc.tensor.matmul(ps[:, :M], xt_s[:, kt, ssl],
                                     wv_s[:, kt, :],
                                     start=(kt == 0), stop=(kt == NKT - 1))
                nc.vector.tensor_copy(out=vv[:, j * nvp + ts_, :],
                                      in_=ps[:, :M])
            return qkt

        def emit_rope(j, qkt):
            tsl = slice(j * tchunk, (j + 1) * tchunk)
            for wi, dest in ((0, qr), (1, kr)):
                for h in range(HPC):
                    src = qkt[:, wi, h, :]
                    shuf = rtmp_pool.tile([P, tchunk], FR, tag="shuf")
                    dst = dest[h][:, tsl]
                    nc.vector.stream_shuffle(out=shuf[:], in_=src,
                                             mask=SWAP_MASK)
                    nc.vector.tensor_mul(out=dst, in0=src,
                                         in1=cos_sb[:, tsl])
                    nc.vector.tensor_mul(out=shuf[:], in0=shuf[:],
                                         in1=sin_sb[:, tsl])
                    nc.vector.tensor_add(out=dst, in0=dst, in1=shuf[:])

        def emit_attn(j, last=False):
            bb, c = divmod(j, ich)
            isl = slice(j * tchunk, (j + 1) * tchunk)
            njt = r_mask * (c + 1)
            outT = outT_pool.tile([P, HPC, tchunk], FR, tag="outT",
                                  name=f"oT{j}")
            for h in range(HPC):
                cs_ps = pC.tile([P, tchunk], F32, tag=f"C{h}",
                                name=f"cs{j}_{h}")
                av_ps = pAV.tile([P, tchunk], F32, tag="AV",
                                 name=f"av{j}_{h}")
                pend = []

                def drain_one():
                    es, plo, pw, pfl, pjt = pend.pop(0)
                    nc.tensor.matmul(cs_ps[:, plo:], ones_s[:],
                                     es[:, :pw], **pfl)
                    nc.tensor.matmul(av_ps[:, plo:],
                                     vv[:, bb * spt + pjt,
                                        h * P:(h + 1) * P],
                                     es[:, :pw], **pfl)

                for jt in range(njt):
                    jsl = slice(bb * s + jt * P, bb * s + (jt + 1) * P)
                    ri = jt - r_mask * c
                    lo = max(ri, 0) * P
                    w = tchunk - lo
                    csl = slice(isl.start + lo, isl.stop)
                    fl = dict(start=(jt == 0), stop=(jt == njt - 1))
                    sc = pS.tile([P, tchunk], F32, tag=f"S{s_idx[0] % 3}",
                                 name=f"sc{j}_{h}_{jt}")
                    s_idx[0] += 1
                    nc.tensor.matmul(sc[:, :w], kr[h][:, jsl],
                                     qr[h][:, csl], start=True, stop=True)
                    es = exps_pool.tile([P, tchunk], FR, tag="es",
                                        name=f"es{j}_{h}_{jt}")
                    nc.scalar.activation(out=es[:, :w], in_=sc[:, :w],
                                         func=EXP, scale=float(SCALE))
                    if ri >= 0:  # diagonal-crossing tile; GpSimd is idle and
                        # keeping masks off the DVE FIFO protects rope/norm
                        nc.gpsimd.tensor_mul(out=es[:, :w], in0=es[:, :w],
                                             in1=mask_s[:, ri, lo:])
                    pend.append((es, lo, w, fl, jt))
                    if len(pend) > 2:
                        drain_one()
                while pend:
                    drain_one()
                # normalize head h
                rcp = rcp_pool.tile([P, tchunk], F32, tag="rcp",
                                    name=f"rcp{j}_{h}")
                nc.vector.reciprocal_approx_fast(out=rcp[:], in_=cs_ps[:])
                nc.vector.tensor_mul(out=outT[:, h, :], in0=av_ps[:],
                                     in1=rcp[:])
            # output projection for the i-chunk
            for tt_ in range(tchunk // P):
                tt0 = isl.start + tt_ * P
                ttsl = slice(tt0, tt0 + P)
                for oc in range(H // 512):
                    osl = slice(oc * 512, (oc + 1) * 512)
                    ps = pS.tile([P, 512], F32, tag=f"S{s_idx[0] % 3}",
                                 name=f"wo{j}_{tt_}_{oc}")
                    s_idx[0] += 1
                    for h in range(HPC):
                        nc.tensor.matmul(
                            ps[:],
                            outT[:, h, tt_ * P:(tt_ + 1) * P],
                            wo_s[:, h, osl],
                            start=(h == 0), stop=(h == HPC - 1))
                    ev = ev_pool.tile([P, 512], FR, tag="ev",
                                      name=f"ev{j}_{tt_}_{oc}")
                    if s_idx[0] % 2:
                        nc.scalar.copy(out=ev[:], in_=ps[:])
                    else:
                        nc.vector.tensor_copy(out=ev[:], in_=ps[:])
                    # stores rotate 2 DMA queues; the final chunk gets the
                    # scalar queue as a 3rd to shorten the closing drain
                    qsel = s_idx[0] % (3 if last else 2)
                    qeng = (nc.sync, nc.gpsimd, nc.scalar)[qsel]
                    qeng.dma_start(out=out[ttsl, osl], in_=ev[:])

        for j in range(nch):
            qkt = emit_proj0() if j == 0 else emit_proj(j)
            if j > 0:
                emit_attn(j - 1)
            emit_rope(j, qkt)
        emit_attn(nch - 1, last=True)

    nc.compile()
    return nc


def make_in_maps(hidden_states, cos, sin, Wq, Wk, Wv, Wo, s=S, b=B,
                 tchunk=TCHUNK):
    t = b * s
    hs = np.asarray(hidden_states, np.float32).reshape(t, H)
    xt = np.ascontiguousarray(hs.T)
    cos2 = np.asarray(cos, np.float32).reshape(s, HD)
    sin2 = np.asarray(sin, np.float32).reshape(s, HD)
    cosP = np.ascontiguousarray(np.tile(cos2[:, PERM].T, (1, b))).astype(BF16)
    sign = np.where(PERM < 64, -1.0, 1.0).astype(np.float32)[:, None]
    sinP = np.ascontiguousarray(
        np.tile(sin2[:, PERM].T * sign, (1, b))).astype(BF16)
    masks_bf = build_masks(tchunk).astype(BF16)
    xt_bf = xt.astype(BF16)
    Wq, Wk, Wv, Wo = (np.asarray(w, np.float32) for w in (Wq, Wk, Wv, Wo))

    in_maps = []
    for c in range(NCORES):
        rows = np.concatenate([(HPC * c + hh) * HD + PERM
                               for hh in range(HPC)])
        sl = slice(c * M, (c + 1) * M)
        in_maps.append({
            "xt": xt_bf,
            "wqt": np.ascontiguousarray(Wq[rows, :].T).astype(BF16),
            "wkt": np.ascontiguousarray(Wk[rows, :].T).astype(BF16),
            "wvt": np.ascontiguousarray(Wv[sl, :].T).astype(BF16),
            "wot": np.ascontiguousarray(Wo[:, sl].T).astype(BF16),
            "cost": cosP,
            "sint": sinP,
            "masks": masks_bf,
        })
    return in_maps


_CACHED_NC = None
_LAST_RESULTS = None


def kernel(hidden_states, cos, sin, Wq, Wk, Wv, Wo):
    global _CACHED_NC, _LAST_RESULTS
    in_maps = make_in_maps(hidden_states, cos, sin, Wq, Wk, Wv, Wo)
    if _CACHED_NC is None:
        _CACHED_NC = build_nc()
    res = run_bass_kernel_spmd(_CACHED_NC, in_maps, core_ids=list(range(NCORES)))
    _LAST_RESULTS = res
    acc = np.zeros((B * S, H), np.float32)
    for r in res.results:
        acc += r["out"].astype(np.float32)
    return acc.reshape(B, S, H)


# revision 27
# speedup vs baseline: 1.0805x; 1.0805x over previous
"""Trainium2 Bass kernel for nn_Attention_3126736192307 — merged pipeline.

Causal multi-head attention with RoPE: B=2, S=2048, H=2048, 16 heads x 128.

Sharding (tensor parallel over heads, 8 cores, 2 heads each):
  - Wq/Wk/Wv column-split (per-head), Wo row-split; each core computes a
    partial [B*S, H] output; the host sums the 8 partials.

v3: merged pipeline at tchunk=512. Chunk-step j emits
  proj(j) -> attn(j-1) -> rope(j)
so attention for chunk j-1 follows chunk j's projections in the PE FIFO
while j's RoPE (DVE) and PSUM evictions (ScalarE) run in their shadows.

PSUM discipline: a matmul accumulation group zeroes its whole 2KB bank at
start, so every concurrently-open group owns a bank. Projections run as
SEQUENTIAL groups (q_h0, q_h1, k_h0, k_h1, v0..v3) rotating 2 banks with
X.T chunk-resident in SBUF; attention processes one head at a time so a
single colsum + a single AV group are open at once (2+1 banks), and
score tiles rotate 3 banks shared with the Wo output tiles. 2+3+2+1 = 8.

Per-core dataflow (all matmuls transpose-free by construction):
  - Host pre-transposes: X.T [H, T], WqT/WkT [H, 256] (head-dim permuted so
    RoPE's rotate_half becomes an intra-quadrant stream_shuffle), WvT [H, 256],
    WoT [256, H], cos/sin [128, T] feature-major bf16 (sin sign-folded).
  - scores.T [j,i] = k.T (lhsT) @ q.T; exp on ScalarE (no max subtraction:
    scores ~N(0,1) after the 1/sqrt(hd) scale); causal block skipping + 0/1
    mask multiply on diagonal-crossing tiles; column sums via ones-matmul on
    TensorE; AV accumulation in PSUM; normalization folded into eviction.
  - out.T (lhsT) @ WoT -> partial [T, H]; PSUM->SBUF evictions alternate
    DVE/ScalarE and the DRAM stores alternate two DMA queues.
"""

import os
import sys

for _p in ("/opt/trn_rl_repo", "/root/.axon_site/_ro/trn_rl_repo"):
    if os.path.isdir(_p) and _p not in sys.path:
        sys.path.append(_p)

from contextlib import ExitStack

import ml_dtypes
import numpy as np

import concourse.bass as bass
import concourse.bacc as bacc
import concourse.tile as tile
from concourse import mybir
from concourse.bass_utils import run_bass_kernel_spmd

B, S, H, NH = 2, 2048, 2048, 16
HD = 128
NCORES = 8
HPC = NH // NCORES            # heads per core = 2
M = HPC * HD                  # 256 output channels per core
SCALE = HD ** -0.5
P = 128                       # partitions
NKT = H // P                  # 16 contraction tiles for projections

F32 = mybir.dt.float32

# head-dim permutation: interleave halves at 16 granularity so the RoPE
# partner (d <-> d+64) sits 16 partitions away inside one 32-part quadrant
PERM = np.concatenate([np.arange(16 * m, 16 * m + 16) + (64 if odd else 0)
                       for m in range(4) for odd in (0, 1)])
SWAP_MASK = [i ^ 16 for i in range(32)]

BF16 = ml_dtypes.bfloat16

TCHUNK = 512


def build_masks(tchunk):
    """0/1 keep-masks for the R diagonal-crossing j-tiles of each i-chunk."""
    r = tchunk // P
    m = np.zeros((r, P, tchunk), np.float32)
    il = np.arange(tchunk)
    for ri in range(r):
        for jl in range(P):
            m[ri, jl, :] = (P * ri + jl <= il).astype(np.float32)
    return m


def build_nc(s=S, b=B, tchunk=TCHUNK, mm_dtype=mybir.dt.bfloat16):
    t = b * s
    nch = t // tchunk             # 8 chunk-steps
    ich = s // tchunk             # 4 attention i-chunks per batch
    r_mask = tchunk // P          # 4 diagonal-crossing tiles per i-chunk
    ntt = t // P                  # 32 token tiles
    nvp = tchunk // P             # 4 v sub-tiles per chunk
    spt = s // P                  # 16 j-tiles per batch

    FR = mm_dtype
    EXP = mybir.ActivationFunctionType.Exp

    nc = bacc.Bacc("TRN2", target_bir_lowering=False, debug=False)

    xt = nc.declare_dram_parameter("xt", [H, t], FR, isOutput=False)
    wqt = nc.declare_dram_parameter("wqt", [H, M], FR, isOutput=False)
    wkt = nc.declare_dram_parameter("wkt", [H, M], FR, isOutput=False)
    wvt = nc.declare_dram_parameter("wvt", [H, M], FR, isOutput=False)
    wot = nc.declare_dram_parameter("wot", [M, H], FR, isOutput=False)
    cost = nc.declare_dram_parameter("cost", [HD, t], FR, isOutput=False)
    sint = nc.declare_dram_parameter("sint", [HD, t], FR, isOutput=False)
    masks = nc.declare_dram_parameter("masks", [r_mask, P, tchunk], FR,
                                      isOutput=False)
    out = nc.declare_dram_parameter("out", [t, H], FR, isOutput=True)

    with tile.TileContext(nc) as tc, ExitStack() as ctx:
        persist = ctx.enter_context(tc.tile_pool(name="persist", bufs=1))

        # merged rope output [P, (q|k), head, t] so rope runs as 4 wide DVE
        # ops instead of 16 narrow ones (per-op SBUF bubble amortizes)
        qkr = persist.tile([P, 2, HPC, t], FR, tag="qkr", name="qkr")
        vv = persist.tile([P, ntt, M], FR, tag="vv")   # v[tt*128+p, d]
        ones_s = persist.tile([P, P], FR, tag="ones")
        nc.vector.memset(ones_s[:], 1.0)
        mask_s = persist.tile([P, r_mask, tchunk], FR, tag="masks")
        wo_s = persist.tile([P, HPC, H], FR, tag="wo")
        cos_sb = persist.tile([P, t], FR, tag="cosb")
        sin_sb = persist.tile([P, t], FR, tag="sinb")
        wq_s = persist.tile([P, NKT, M], FR, tag="wq")
        wk_s = persist.tile([P, NKT, M], FR, tag="wk")
        wv_s = persist.tile([P, NKT, M], FR, tag="wv")

        xt_pool = ctx.enter_context(tc.tile_pool(name="xtp", bufs=2))
        qkt_pool = ctx.enter_context(tc.tile_pool(name="qkt", bufs=2))
        rtmp_pool = ctx.enter_context(tc.tile_pool(name="rtmp", bufs=3))
        exps_pool = ctx.enter_context(tc.tile_pool(name="exps", bufs=6))
        rcp_pool = ctx.enter_context(tc.tile_pool(name="rcp", bufs=2))
        outT_pool = ctx.enter_context(tc.tile_pool(name="outT", bufs=2))
        ev_pool = ctx.enter_context(tc.tile_pool(name="evp", bufs=8))
        # PSUM: 8 banks: PR0 PR1 (sequential projection groups), S0 S1 S2
        # (score j-tiles + Wo outputs), C0 C1 (per-head colsums), AV
        pPR = ctx.enter_context(tc.tile_pool(name="pPR", bufs=1, space="PSUM"))
        pS = ctx.enter_context(tc.tile_pool(name="pS", bufs=1, space="PSUM"))
        pC = ctx.enter_context(tc.tile_pool(name="pC", bufs=1, space="PSUM"))
        pAV = ctx.enter_context(tc.tile_pool(name="pAV", bufs=1, space="PSUM"))

        # upfront loads on the gpsimd SWDGE queue in order of first use;
        # kt0 slivers first so the opening matmuls aren't gated on bulk DMAs
        for w_s, wsrc in ((wq_s, wqt), (wk_s, wkt), (wv_s, wvt)):
            nc.gpsimd.dma_start(
                out=w_s[:, 0:1, :],
                in_=wsrc[0:P, :].rearrange("(k p) m -> p k m", p=P))
        KWG = 5                     # weight k-tiles per bulk DMA (1+5+5+5)
        for lo_kt in range(1, NKT, KWG):
            hi_kt = min(lo_kt + KWG, NKT)
            gsl = slice(lo_kt * P, hi_kt * P)
            for w_s, wsrc in ((wq_s, wqt), (wk_s, wkt), (wv_s, wvt)):
                nc.gpsimd.dma_start(
                    out=w_s[:, lo_kt:hi_kt, :],
                    in_=wsrc[gsl, :].rearrange("(k p) m -> p k m", p=P))
        nc.gpsimd.dma_start(out=cos_sb[:], in_=cost[:, :])
        nc.gpsimd.dma_start(out=sin_sb[:], in_=sint[:, :])
        nc.gpsimd.dma_start(out=mask_s[:],
                            in_=masks.rearrange("r p n -> p r n"))
        nc.gpsimd.dma_start(out=wo_s[:],
                            in_=wot.rearrange("(mt p) o -> p mt o", p=P))

        pr_idx = [0]                  # rotating projection bank
        s_idx = [0]                   # rotating score/wo bank

        def emit_proj0():
            """Chunk 0: kt-interleaved groups across all 8 banks so the PE
            consumes each xt k-tile as it lands (startup is DMA-bound)."""
            j = 0
            tsl = slice(0, tchunk)
            xt_s = xt_pool.tile([P, NKT, tchunk], FR, tag="xt", name="xt0")
            nc.sync.dma_start(
                out=xt_s[:, 0:1, :],
                in_=xt[0:P, tsl].rearrange("(k p) t -> p k t", p=P))
            nc.sync.dma_start(
                out=xt_s[:, 1:4, :],
                in_=xt[P:4 * P, tsl].rearrange("(k p) t -> p k t", p=P))
            for g in range(1, 4):
                nc.sync.dma_start(
                    out=xt_s[:, 4 * g:4 * g + 4, :],
                    in_=xt[4 * g * P:(4 * g + 4) * P, tsl].rearrange(
                        "(k p) t -> p k t", p=P))
            qk_ps = [pPR.tile([P, tchunk], F32, tag="PR0", name="q0_0"),
                     pPR.tile([P, tchunk], F32, tag="PR1", name="q0_1"),
                     pS.tile([P, tchunk], F32, tag="S0", name="k0_0"),
                     pS.tile([P, tchunk], F32, tag="S1", name="k0_1")]
            v_ps = [pS.tile([P, tchunk], F32, tag="S2", name="v0_0"),
                    pC.tile([P, tchunk], F32, tag="C0", name="v0_1"),
                    pC.tile([P, tchunk], F32, tag="C1", name="v0_2"),
                    pAV.tile([P, tchunk], F32, tag="AV", name="v0_3")]
            for kt in range(NKT):
                fl = dict(start=(kt == 0), stop=(kt == NKT - 1))
                for wi, w_s in ((0, wq_s), (1, wk_s)):
                    for h in range(HPC):
                        msl = slice(h * P, (h + 1) * P)
                        nc.tensor.matmul(qk_ps[wi * HPC + h][:],
                                         w_s[:, kt, msl], xt_s[:, kt, :],
                                         **fl)
                for ts_ in range(nvp):
                    ssl = slice(ts_ * P, (ts_ + 1) * P)
                    nc.tensor.matmul(v_ps[ts_][:, :M], xt_s[:, kt, ssl],
                                     wv_s[:, kt, :], **fl)
            qkt = qkt_pool.tile([P, 2, HPC, tchunk], FR, tag="qkt",
                                name="qkt0")
            for wi in range(2):
                for h in range(HPC):
                    nc.scalar.copy(out=qkt[:, wi, h, :],
                                   in_=qk_ps[wi * HPC + h][:])
            for ts_ in range(nvp):
                nc.vector.tensor_copy(out=vv[:, ts_, :],
                                      in_=v_ps[ts_][:, :M])
            return qkt

        def emit_proj(j):
            tsl = slice(j * tchunk, (j + 1) * tchunk)
            # chunk-resident X.T: [P, NKT, tchunk] bf16 (16KB/partition)
            xt_s = xt_pool.tile([P, NKT, tchunk], FR, tag="xt",
                                name=f"xt{j}")
            for lo_kt in range(0, NKT, 8):
                nc.sync.dma_start(
                    out=xt_s[:, lo_kt:lo_kt + 8, :],
                    in_=xt[lo_kt * P:(lo_kt + 8) * P, tsl].rearrange(
                        "(k p) t -> p k t", p=P))
            qkt = qkt_pool.tile([P, 2, HPC, tchunk], FR, tag="qkt",
                                name=f"qkt{j}")
            # sequential accumulation groups, 2-bank rotation; evictions
            # (ScalarE q/k, DVE v) chase the groups
            for wi, w_s in ((0, wq_s), (1, wk_s)):
                for h in range(HPC):
                    ps = pPR.tile([P, tchunk], F32,
                                  tag=f"PR{pr_idx[0] % 2}",
                                  name=f"p{j}_{wi}_{h}")
                    pr_idx[0] += 1
                    msl = slice(h * P, (h + 1) * P)
                    for kt in range(NKT):
                        nc.tensor.matmul(ps[:], w_s[:, kt, msl],
                                         xt_s[:, kt, :],
                                         start=(kt == 0),
                                         stop=(kt == NKT - 1))
                    nc.scalar.copy(out=qkt[:, wi, h, :], in_=ps[:])
            for ts_ in range(nvp):
                ps = pPR.tile([P, tchunk], F32, tag=f"PR{pr_idx[0] % 2}",
                              name=f"pv{j}_{ts_}")
                pr_idx[0] += 1
                ssl = slice(ts_ * P, (ts_ + 1) * P)
                for kt in range(NKT):
                    nc.tensor.matmul(ps[:, :M], xt_s[:, kt, ssl],
                                     wv_s[:, kt, :],
                                     start=(kt == 0), stop=(kt == NKT - 1))
                nc.vector.tensor_copy(out=vv[:, j * nvp + ts_, :],
                                      in_=ps[:, :M])
            return qkt

        def emit_rope(j, qkt):
            tsl = slice(j * tchunk, (j + 1) * tchunk)
            for wi in range(2):
                for h in range(HPC):
                    src = qkt[:, wi, h, :]
                    shuf = rtmp_pool.tile([P, tchunk], FR, tag="shuf")
                    dst = qkr[:, wi, h, tsl]
                    nc.vector.stream_shuffle(out=shuf[:], in_=src,
                                             mask=SWAP_MASK)
                    nc.vector.tensor_mul(out=dst, in0=src,
                                         in1=cos_sb[:, tsl])
                    nc.vector.tensor_mul(out=shuf[:], in0=shuf[:],
                                         in1=sin_sb[:, tsl])
                    nc.vector.tensor_add(out=dst, in0=dst, in1=shuf[:])

        def emit_attn(j, last=False):
            bb, c = divmod(j, ich)
            isl = slice(j * tchunk, (j + 1) * tchunk)
            njt = r_mask * (c + 1)
            outT = outT_pool.tile([P, HPC, tchunk], FR, tag="outT",
                                  name=f"oT{j}")

            def emit_wo_tt(tt_):
                tt0 = isl.start + tt_ * P
                ttsl = slice(tt0, tt0 + P)
                for oc in range(H // 512):
                    osl = slice(oc * 512, (oc + 1) * 512)
                    ps = pS.tile([P, 512], F32, tag=f"S{s_idx[0] % 3}",
                                 name=f"wo{j}_{tt_}_{oc}")
                    s_idx[0] += 1
                    for hh in range(HPC):
                        nc.tensor.matmul(
                            ps[:],
                            outT[:, hh, tt_ * P:(tt_ + 1) * P],
                            wo_s[:, hh, osl],
                            start=(hh == 0), stop=(hh == HPC - 1))
                    ev = ev_pool.tile([P, 512], FR, tag="ev",
                                      name=f"ev{j}_{tt_}_{oc}")
                    if s_idx[0] % 2:
                        nc.scalar.copy(out=ev[:], in_=ps[:])
                    else:
                        nc.vector.tensor_copy(out=ev[:], in_=ps[:])
                    # stores rotate 2 DMA queues
                    qeng = (nc.sync, nc.gpsimd)[s_idx[0] % 2]
                    qeng.dma_start(out=out[ttsl, osl], in_=ev[:])

            for h in range(HPC):
                cs_ps = pC.tile([P, tchunk], F32, tag=f"C{h}",
                                name=f"cs{j}_{h}")
                av_ps = pAV.tile([P, tchunk], F32, tag="AV",
                                 name=f"av{j}_{h}")
                pend = []

                def drain_one():
                    es, plo, pw, pfl, pjt = pend.pop(0)
                    nc.tensor.matmul(cs_ps[:, plo:], ones_s[:],
                                     es[:, :pw], **pfl)
                    nc.tensor.matmul(av_ps[:, plo:],
                                     vv[:, bb * spt + pjt,
                                        h * P:(h + 1) * P],
                                     es[:, :pw], **pfl)
                    if last and h == HPC - 1 and pjt >= r_mask * c:
                        # final chunk: columns [tt*128,(tt+1)*128) of cs/av
                        # are final once diagonal tile ri=tt has drained, so
                        # normalize + project that token sub-tile NOW and
                        # pipeline the epilogue into the drain phase
                        tt_ = pjt - r_mask * c
                        ttc = slice(tt_ * P, (tt_ + 1) * P)
                        rcpn = rcp_pool.tile([P, P], F32, tag="rcpn",
                                             name=f"rcpn{j}_{tt_}")
                        nc.vector.reciprocal_approx_fast(out=rcpn[:],
                                                         in_=cs_ps[:, ttc])
                        nc.vector.tensor_mul(out=outT[:, h, ttc],
                                             in0=av_ps[:, ttc],
                                             in1=rcpn[:])
                        emit_wo_tt(tt_)

                for jt in range(njt):
                    jsl = slice(bb * s + jt * P, bb * s + (jt + 1) * P)
                    ri = jt - r_mask * c
                    lo = max(ri, 0) * P
                    w = tchunk - lo
                    csl = slice(isl.start + lo, isl.stop)
                    fl = dict(start=(jt == 0), stop=(jt == njt - 1))
                    sc = pS.tile([P, tchunk], F32, tag=f"S{s_idx[0] % 3}",
                                 name=f"sc{j}_{h}_{jt}")
                    s_idx[0] += 1
                    nc.tensor.matmul(sc[:, :w], qkr[:, 1, h, jsl],
                                     qkr[:, 0, h, csl], start=True,
                                     stop=True)
                    es = exps_pool.tile([P, tchunk], FR, tag="es",
                                        name=f"es{j}_{h}_{jt}")
                    nc.scalar.activation(out=es[:, :w], in_=sc[:, :w],
                                         func=EXP, scale=float(SCALE))
                    if ri >= 0:  # diagonal-crossing tile
                        nc.vector.tensor_mul(out=es[:, :w], in0=es[:, :w],
                                             in1=mask_s[:, ri, lo:])
                    pend.append((es, lo, w, fl, jt))
                    if len(pend) > 2:
                        drain_one()
                while pend:
                    drain_one()
                if not (last and h == HPC - 1):
                    # normalize head h (wide)
                    rcp = rcp_pool.tile([P, tchunk], F32, tag="rcp",
                                        name=f"rcp{j}_{h}")
                    nc.vector.reciprocal_approx_fast(out=rcp[:],
                                                     in_=cs_ps[:])
                    nc.vector.tensor_mul(out=outT[:, h, :], in0=av_ps[:],
                                         in1=rcp[:])
            if not last:
                # output projection for the i-chunk
                for tt_ in range(tchunk // P):
                    emit_wo_tt(tt_)

        for j in range(nch):
            qkt = emit_proj0() if j == 0 else emit_proj(j)
            if j > 0:
                emit_attn(j - 1)
            emit_rope(j, qkt)
        emit_attn(nch - 1, last=True)

    nc.compile()
    return nc


def make_in_maps(hidden_states, cos, sin, Wq, Wk, Wv, Wo, s=S, b=B,
                 tchunk=TCHUNK):
    t = b * s
    hs = np.asarray(hidden_states, np.float32).reshape(t, H)
    xt = np.ascontiguousarray(hs.T)
    cos2 = np.asarray(cos, np.float32).reshape(s, HD)
    sin2 = np.asarray(sin, np.float32).reshape(s, HD)
    cosP = np.ascontiguousarray(np.tile(cos2[:, PERM].T, (1, b))).astype(BF16)
    sign = np.where(PERM < 64, -1.0, 1.0).astype(np.float32)[:, None]
    sinP = np.ascontiguousarray(
        np.tile(sin2[:, PERM].T * sign, (1, b))).astype(BF16)
    masks_bf = build_masks(tchunk).astype(BF16)
    xt_bf = xt.astype(BF16)
    Wq, Wk, Wv, Wo = (np.asarray(w, np.float32) for w in (Wq, Wk, Wv, Wo))

    in_maps = []
    for c in range(NCORES):
        rows = np.concatenate([(HPC * c + hh) * HD + PERM
                               for hh in range(HPC)])
        sl = slice(c * M, (c + 1) * M)
        in_maps.append({
            "xt": xt_bf,
            "wqt": np.ascontiguousarray(Wq[rows, :].T).astype(BF16),
            "wkt": np.ascontiguousarray(Wk[rows, :].T).astype(BF16),
            "wvt": np.ascontiguousarray(Wv[sl, :].T).astype(BF16),
            "wot": np.ascontiguousarray(Wo[:, sl].T).astype(BF16),
            "cost": cosP,
            "sint": sinP,
            "masks": masks_bf,
        })
    return in_maps


_CACHED_NC = None
_LAST_RESULTS = None


def kernel(hidden_states, cos, sin, Wq, Wk, Wv, Wo):
    global _CACHED_NC, _LAST_RESULTS
    in_maps = make_in_maps(hidden_states, cos, sin, Wq, Wk, Wv, Wo)
    if _CACHED_NC is None:
        _CACHED_NC = build_nc()
    res = run_bass_kernel_spmd(_CACHED_NC, in_maps, core_ids=list(range(NCORES)))
    _LAST_RESULTS = res
    acc = np.zeros((B * S, H), np.float32)
    for r in res.results:
        acc += r["out"].astype(np.float32)
    return acc.reshape(B, S, H)


# revision 40
# speedup vs baseline: 1.0988x; 1.0169x over previous
"""Trainium2 Bass kernel for nn_Attention_3126736192307 — merged pipeline.

Causal multi-head attention with RoPE: B=2, S=2048, H=2048, 16 heads x 128.

Sharding (tensor parallel over heads, 8 cores, 2 heads each):
  - Wq/Wk/Wv column-split (per-head), Wo row-split; each core computes a
    partial [B*S, H] output; the host sums the 8 partials.

Merged single pipeline at tchunk=512 (8 chunk-steps, batch-major). Step j
emits  proj(j) -> attn(j-1) -> rope(j)  so attention for chunk j-1 follows
chunk j's projections in the PE FIFO while chunk j's RoPE (DVE) and PSUM
evictions (ScalarE) run in their shadows - the PE never waits for a phase
transition.

PSUM discipline: a matmul accumulation group zeroes its whole 2KB bank at
start, so every concurrently-open group owns a bank. Projections run as
SEQUENTIAL groups (q_h0, q_h1, k_h0, k_h1, v0..v3) rotating 2 banks with
X.T chunk-resident in SBUF; attention processes one head at a time so a
single colsum + a single AV group are open at once, and score tiles
rotate 3 banks shared with the Wo output tiles. 2(PR)+3(S)+2(C)+1(AV)=8.

Engine placement (per-FIFO coupling is what matters, not just totals):
  - ScalarE: exp + q/k PSUM evictions (idle during proj) + half the Wo
    output evictions.  DVE: rope, masks, v eviction, normalization, the
    other half of Wo evictions.  h0's AV bank is released with a single
    raw cast so h1's first drain is not gated on rcp+norm.
  - Weight/cos/sin/mask DMAs ride the gpsimd SWDGE queue (independent
    path); x.T and half the output stores ride the sync HWDGE queue (the
    scalar HWDGE shares bandwidth with sync - keep it free); the other
    half of the stores ride gpsimd.  kt0 slivers load first so the
    opening matmuls are not gated on bulk transfers.
  - Final chunk: Wo is projected per 128-token sub-tile as soon as its
    cs/av columns finalize (after diagonal tile ri=tt drains), hiding the
    epilogue inside the drain phase.

Per-core dataflow (all matmuls transpose-free by construction):
  - Host pre-transposes: X.T [H, T], WqT/WkT [H, 256] (head-dim permuted so
    RoPE's rotate_half becomes an intra-quadrant stream_shuffle), WvT [H, 256],
    WoT [256, H], cos/sin [128, T] feature-major bf16 (sin sign-folded).
  - scores.T [j,i] = k.T (lhsT) @ q.T; exp on ScalarE (no max subtraction:
    scores ~N(0,1) after the 1/sqrt(hd) scale); causal block skipping + 0/1
    mask multiply on diagonal-crossing tiles; column sums via ones-matmul on
    TensorE; AV accumulation in PSUM; normalization folded into eviction.
  - out.T (lhsT) @ WoT -> partial [T, H]; the host sums the 8 partials.

Matmuls run in bf16 (1 PE cycle/row; fp32 is 4x). fp8 DoubleRow (~1.44x)
was evaluated and rejected: e4m3 quantization of any main-path operand
costs ~2.5-6% relative error vs the 2e-2 budget (measured in numpy).
"""

import os
import sys

for _p in ("/opt/trn_rl_repo", "/root/.axon_site/_ro/trn_rl_repo"):
    if os.path.isdir(_p) and _p not in sys.path:
        sys.path.append(_p)

from contextlib import ExitStack

import ml_dtypes
import numpy as np

import concourse.bass as bass
import concourse.bacc as bacc
import concourse.tile as tile
from concourse import mybir
from concourse.bass_utils import run_bass_kernel_spmd

B, S, H, NH = 2, 2048, 2048, 16
HD = 128
NCORES = 8
HPC = NH // NCORES            # heads per core = 2
M = HPC * HD                  # 256 output channels per core
SCALE = HD ** -0.5
P = 128                       # partitions
NKT = H // P                  # 16 contraction tiles for projections

F32 = mybir.dt.float32

# head-dim permutation: interleave halves at 16 granularity so the RoPE
# partner (d <-> d+64) sits 16 partitions away inside one 32-part quadrant
PERM = np.concatenate([np.arange(16 * m, 16 * m + 16) + (64 if odd else 0)
                       for m in range(4) for odd in (0, 1)])
SWAP_MASK = [i ^ 16 for i in range(32)]

BF16 = ml_dtypes.bfloat16

TCHUNK = 512


def build_masks(tchunk):
    """0/1 keep-masks for the R diagonal-crossing j-tiles of each i-chunk."""
    r = tchunk // P
    m = np.zeros((r, P, tchunk), np.float32)
    il = np.arange(tchunk)
    for ri in range(r):
        for jl in range(P):
            m[ri, jl, :] = (P * ri + jl <= il).astype(np.float32)
    return m


def build_nc(s=S, b=B, tchunk=TCHUNK, mm_dtype=mybir.dt.bfloat16,
             tail_pipeline=True):
    # tail_pipeline: the final chunk's Wo epilogue reads cs/av sub-ranges
    # whose drains have completed but whose PSUM accumulation group is
    # still open. Correct on HW (per-element has_written; Tile range-tracks
    # the deps) but CoreSim rejects mid-group reads, so sim runs disable it.
    t = b * s
    nch = t // tchunk             # 8 chunk-steps
    ich = s // tchunk             # 4 attention i-chunks per batch
    r_mask = tchunk // P          # 4 diagonal-crossing tiles per i-chunk
    ntt = t // P                  # 32 token tiles
    nvp = tchunk // P             # 4 v sub-tiles per chunk
    spt = s // P                  # 16 j-tiles per batch

    FR = mm_dtype
    EXP = mybir.ActivationFunctionType.Exp

    nc = bacc.Bacc("TRN2", target_bir_lowering=False, debug=False)

    xt = nc.declare_dram_parameter("xt", [H, t], FR, isOutput=False)
    wqt = nc.declare_dram_parameter("wqt", [H, M], FR, isOutput=False)
    wkt = nc.declare_dram_parameter("wkt", [H, M], FR, isOutput=False)
    wvt = nc.declare_dram_parameter("wvt", [H, M], FR, isOutput=False)
    wot = nc.declare_dram_parameter("wot", [M, H], FR, isOutput=False)
    cost = nc.declare_dram_parameter("cost", [HD, t], FR, isOutput=False)
    sint = nc.declare_dram_parameter("sint", [HD, t], FR, isOutput=False)
    masks = nc.declare_dram_parameter("masks", [r_mask, P, tchunk], FR,
                                      isOutput=False)
    out = nc.declare_dram_parameter("out", [t, H], FR, isOutput=True)

    with tile.TileContext(nc) as tc, ExitStack() as ctx:
        persist = ctx.enter_context(tc.tile_pool(name="persist", bufs=1))

        # merged rope output [P, (q|k), head, t] so rope runs as 4 wide DVE
        # ops instead of 16 narrow ones (per-op SBUF bubble amortizes)
        qkr = persist.tile([P, 2, HPC, t], FR, tag="qkr", name="qkr")
        vv = persist.tile([P, ntt, M], FR, tag="vv")   # v[tt*128+p, d]
        ones_s = persist.tile([P, P], FR, tag="ones")
        nc.vector.memset(ones_s[:], 1.0)
        mask_s = persist.tile([P, r_mask, tchunk], FR, tag="masks")
        wo_s = persist.tile([P, HPC, H], FR, tag="wo")
        cos_sb = persist.tile([P, t], FR, tag="cosb")
        sin_sb = persist.tile([P, t], FR, tag="sinb")
        wq_s = persist.tile([P, NKT, M], FR, tag="wq")
        wk_s = persist.tile([P, NKT, M], FR, tag="wk")
        wv_s = persist.tile([P, NKT, M], FR, tag="wv")

        xt_pool = ctx.enter_context(tc.tile_pool(name="xtp", bufs=2))
        qkt_pool = ctx.enter_context(tc.tile_pool(name="qkt", bufs=2))
        rtmp_pool = ctx.enter_context(tc.tile_pool(name="rtmp", bufs=3))
        exps_pool = ctx.enter_context(tc.tile_pool(name="exps", bufs=6))
        rcp_pool = ctx.enter_context(tc.tile_pool(name="rcp", bufs=2))
        outT_pool = ctx.enter_context(tc.tile_pool(name="outT", bufs=2))
        ev_pool = ctx.enter_context(tc.tile_pool(name="evp", bufs=8))
        # PSUM: 8 banks: PR0 PR1 (sequential projection groups), S0 S1 S2
        # (score j-tiles + Wo outputs), C0 C1 (per-head colsums), AV
        pPR = ctx.enter_context(tc.tile_pool(name="pPR", bufs=1, space="PSUM"))
        pS = ctx.enter_context(tc.tile_pool(name="pS", bufs=1, space="PSUM"))
        pC = ctx.enter_context(tc.tile_pool(name="pC", bufs=1, space="PSUM"))
        pAV = ctx.enter_context(tc.tile_pool(name="pAV", bufs=1, space="PSUM"))

        # upfront loads in order of first use, spread over the gpsimd and
        # scalar SWDGE queues (sync carries chunk-0's xt): startup is
        # DMA-bandwidth-bound, one queue sustains only ~70GB/s.
        # kt0 slivers first so the opening matmuls aren't gated on bulk DMAs.
        KWG = 5                     # weight k-tiles per bulk DMA (1+5+5+5)
        for w_s, wsrc in ((wq_s, wqt), (wk_s, wkt), (wv_s, wvt)):
            nc.gpsimd.dma_start(
                out=w_s[:, 0:1, :],
                in_=wsrc[0:P, :].rearrange("(k p) m -> p k m", p=P))
        for lo_kt in range(1, NKT, KWG):
            hi_kt = min(lo_kt + KWG, NKT)
            gsl = slice(lo_kt * P, hi_kt * P)
            for w_s, wsrc in ((wq_s, wqt), (wk_s, wkt), (wv_s, wvt)):
                nc.gpsimd.dma_start(
                    out=w_s[:, lo_kt:hi_kt, :],
                    in_=wsrc[gsl, :].rearrange("(k p) m -> p k m", p=P))
        nc.gpsimd.dma_start(out=cos_sb[:], in_=cost[:, :])
        nc.gpsimd.dma_start(out=sin_sb[:], in_=sint[:, :])
        nc.gpsimd.dma_start(out=mask_s[:],
                            in_=masks.rearrange("r p n -> p r n"))
        nc.gpsimd.dma_start(out=wo_s[:],
                            in_=wot.rearrange("(mt p) o -> p mt o", p=P))

        pr_idx = [0]                  # rotating projection bank
        s_idx = [0]                   # rotating score/wo bank

        def emit_proj(j):
            tsl = slice(j * tchunk, (j + 1) * tchunk)
            # chunk-resident X.T: [P, NKT, tchunk] bf16 (16KB/partition)
            xt_s = xt_pool.tile([P, NKT, tchunk], FR, tag="xt",
                                name=f"xt{j}")
            if j == 0:
                # kt0 sliver first so the opening matmul isn't DMA-gated
                nc.sync.dma_start(
                    out=xt_s[:, 0:1, :],
                    in_=xt[0:P, tsl].rearrange("(k p) t -> p k t", p=P))
                for lo_kt in range(1, NKT, KWG):
                    hi_kt = min(lo_kt + KWG, NKT)
                    nc.sync.dma_start(
                        out=xt_s[:, lo_kt:hi_kt, :],
                        in_=xt[lo_kt * P:hi_kt * P, tsl].rearrange(
                            "(k p) t -> p k t", p=P))
            else:
                for lo_kt in range(0, NKT, 8):
                    nc.sync.dma_start(
                        out=xt_s[:, lo_kt:lo_kt + 8, :],
                        in_=xt[lo_kt * P:(lo_kt + 8) * P, tsl].rearrange(
                            "(k p) t -> p k t", p=P))
            qkt = qkt_pool.tile([P, 2, HPC, tchunk], FR, tag="qkt",
                                name=f"qkt{j}")
            # sequential accumulation groups, 2-bank rotation; evictions
            # (ScalarE q/k, DVE v) chase the groups
            for wi, w_s in ((0, wq_s), (1, wk_s)):
                for h in range(HPC):
                    ps = pPR.tile([P, tchunk], F32,
                                  tag=f"PR{pr_idx[0] % 2}",
                                  name=f"p{j}_{wi}_{h}")
                    pr_idx[0] += 1
                    msl = slice(h * P, (h + 1) * P)
                    for kt in range(NKT):
                        nc.tensor.matmul(ps[:], w_s[:, kt, msl],
                                         xt_s[:, kt, :],
                                         start=(kt == 0),
                                         stop=(kt == NKT - 1))
                    # ScalarE: it is idle during projections, while DVE is
                    # still busy with the previous chunk's rope at this point
                    nc.scalar.copy(out=qkt[:, wi, h, :], in_=ps[:])
            for ts_ in range(nvp):
                ps = pPR.tile([P, tchunk], F32, tag=f"PR{pr_idx[0] % 2}",
                              name=f"pv{j}_{ts_}")
                pr_idx[0] += 1
                ssl = slice(ts_ * P, (ts_ + 1) * P)
                for kt in range(NKT):
                    nc.tensor.matmul(ps[:, :M], xt_s[:, kt, ssl],
                                     wv_s[:, kt, :],
                                     start=(kt == 0), stop=(kt == NKT - 1))
                nc.vector.tensor_copy(out=vv[:, j * nvp + ts_, :],
                                      in_=ps[:, :M])
            return qkt

        def emit_rope(j, qkt):
            tsl = slice(j * tchunk, (j + 1) * tchunk)
            for wi in range(2):
                for h in range(HPC):
                    src = qkt[:, wi, h, :]
                    shuf = rtmp_pool.tile([P, tchunk], FR, tag="shuf")
                    dst = qkr[:, wi, h, tsl]
                    nc.vector.stream_shuffle(out=shuf[:], in_=src,
                                             mask=SWAP_MASK)
                    nc.vector.tensor_mul(out=dst, in0=src,
                                         in1=cos_sb[:, tsl])
                    nc.vector.tensor_mul(out=shuf[:], in0=shuf[:],
                                         in1=sin_sb[:, tsl])
                    nc.vector.tensor_add(out=dst, in0=dst, in1=shuf[:])

        def emit_attn(j, last=False):
            last = last and tail_pipeline
            bb, c = divmod(j, ich)
            isl = slice(j * tchunk, (j + 1) * tchunk)
            njt = r_mask * (c + 1)
            outT = outT_pool.tile([P, HPC, tchunk], FR, tag="outT",
                                  name=f"oT{j}")

            def emit_wo_tt(tt_):
                tt0 = isl.start + tt_ * P
                ttsl = slice(tt0, tt0 + P)
                for oc in range(H // 512):
                    osl = slice(oc * 512, (oc + 1) * 512)
                    ps = pS.tile([P, 512], F32, tag=f"S{s_idx[0] % 3}",
                                 name=f"wo{j}_{tt_}_{oc}")
                    s_idx[0] += 1
                    for hh in range(HPC):
                        nc.tensor.matmul(
                            ps[:],
                            outT[:, hh, tt_ * P:(tt_ + 1) * P],
                            wo_s[:, hh, osl],
                            start=(hh == 0), stop=(hh == HPC - 1))
                    ev = ev_pool.tile([P, 512], FR, tag="ev",
                                      name=f"ev{j}_{tt_}_{oc}")
                    if s_idx[0] % 2:
                        nc.scalar.copy(out=ev[:], in_=ps[:])
                    else:
                        nc.vector.tensor_copy(out=ev[:], in_=ps[:])
                    # stores rotate 2 DMA queues
                    qeng = (nc.sync, nc.gpsimd)[s_idx[0] % 2]
                    qeng.dma_start(out=out[ttsl, osl], in_=ev[:])

            for h in range(HPC):
                cs_ps = pC.tile([P, tchunk], F32, tag=f"C{h}",
                                name=f"cs{j}_{h}")
                av_ps = pAV.tile([P, tchunk], F32, tag="AV",
                                 name=f"av{j}_{h}")
                pend = []

                def drain_one():
                    es, plo, pw, pfl, pjt = pend.pop(0)
                    nc.tensor.matmul(cs_ps[:, plo:], ones_s[:],
                                     es[:, :pw], **pfl)
                    nc.tensor.matmul(av_ps[:, plo:],
                                     vv[:, bb * spt + pjt,
                                        h * P:(h + 1) * P],
                                     es[:, :pw], **pfl)
                    if last and h == HPC - 1 and pjt >= r_mask * c:
                        # final chunk: columns [tt*128,(tt+1)*128) of cs/av
                        # are final once diagonal tile ri=tt has drained, so
                        # normalize + project that token sub-tile NOW and
                        # pipeline the epilogue into the drain phase
                        tt_ = pjt - r_mask * c
                        ttc = slice(tt_ * P, (tt_ + 1) * P)
                        rcpn = rcp_pool.tile([P, P], F32, tag="rcpn",
                                             name=f"rcpn{j}_{tt_}")
                        nc.vector.reciprocal_approx_fast(out=rcpn[:],
                                                         in_=cs_ps[:, ttc])
                        nc.vector.tensor_mul(out=outT[:, h, ttc],
                                             in0=av_ps[:, ttc],
                                             in1=rcpn[:])
                        emit_wo_tt(tt_)

                for jt in range(njt):
                    jsl = slice(bb * s + jt * P, bb * s + (jt + 1) * P)
                    ri = jt - r_mask * c
                    lo = max(ri, 0) * P
                    w = tchunk - lo
                    csl = slice(isl.start + lo, isl.stop)
                    fl = dict(start=(jt == 0), stop=(jt == njt - 1))
                    sc = pS.tile([P, tchunk], F32, tag=f"S{s_idx[0] % 3}",
                                 name=f"sc{j}_{h}_{jt}")
                    s_idx[0] += 1
                    nc.tensor.matmul(sc[:, :w], qkr[:, 1, h, jsl],
                                     qkr[:, 0, h, csl], start=True,
                                     stop=True)
                    es = exps_pool.tile([P, tchunk], FR, tag="es",
                                        name=f"es{j}_{h}_{jt}")
                    nc.scalar.activation(out=es[:, :w], in_=sc[:, :w],
                                         func=EXP, scale=float(SCALE))
                    if ri >= 0:  # diagonal-crossing tile
                        nc.vector.tensor_mul(out=es[:, :w], in0=es[:, :w],
                                             in1=mask_s[:, ri, lo:])
                    pend.append((es, lo, w, fl, jt))
                    if len(pend) > 2:
                        drain_one()
                while pend:
                    drain_one()
                if h < HPC - 1:
                    # h0/h1 share the AV bank: release it with a single raw
                    # cast so h1's first drain isn't gated on rcp+norm; the
                    # normalization then runs off-path from the SBUF copy
                    avraw = rcp_pool.tile([P, tchunk], FR, tag="avraw",
                                          name=f"avr{j}_{h}")
                    nc.vector.tensor_copy(out=avraw[:], in_=av_ps[:])
                    rcp = rcp_pool.tile([P, tchunk], F32, tag="rcp",
                                        name=f"rcp{j}_{h}")
                    nc.vector.reciprocal_approx_fast(out=rcp[:],
                                                     in_=cs_ps[:])
                    nc.vector.tensor_mul(out=outT[:, h, :], in0=avraw[:],
                                         in1=rcp[:])
                elif not last:
                    # last head: normalize straight from PSUM (wide)
                    rcp = rcp_pool.tile([P, tchunk], F32, tag="rcp",
                                        name=f"rcp{j}_{h}")
                    nc.vector.reciprocal_approx_fast(out=rcp[:],
                                                     in_=cs_ps[:])
                    nc.vector.tensor_mul(out=outT[:, h, :], in0=av_ps[:],
                                         in1=rcp[:])
            if not last:
                # output projection for the i-chunk
                for tt_ in range(tchunk // P):
                    emit_wo_tt(tt_)

        for j in range(nch):
            qkt = emit_proj(j)
            if j > 0:
                emit_attn(j - 1)
            emit_rope(j, qkt)
        emit_attn(nch - 1, last=True)

    nc.compile()
    return nc


def make_in_maps(hidden_states, cos, sin, Wq, Wk, Wv, Wo, s=S, b=B,
                 tchunk=TCHUNK):
    t = b * s
    hs = np.asarray(hidden_states, np.float32).reshape(t, H)
    xt = np.ascontiguousarray(hs.T)
    cos2 = np.asarray(cos, np.float32).reshape(s, HD)
    sin2 = np.asarray(sin, np.float32).reshape(s, HD)
    cosP = np.ascontiguousarray(np.tile(cos2[:, PERM].T, (1, b))).astype(BF16)
    sign = np.where(PERM < 64, -1.0, 1.0).astype(np.float32)[:, None]
    sinP = np.ascontiguousarray(
        np.tile(sin2[:, PERM].T * sign, (1, b))).astype(BF16)
    masks_bf = build_masks(tchunk).astype(BF16)
    xt_bf = xt.astype(BF16)
    Wq, Wk, Wv, Wo = (np.asarray(w, np.float32) for w in (Wq, Wk, Wv, Wo))

    in_maps = []
    for c in range(NCORES):
        rows = np.concatenate([(HPC * c + hh) * HD + PERM
                               for hh in range(HPC)])
        sl = slice(c * M, (c + 1) * M)
        in_maps.append({
            "xt": xt_bf,
            "wqt": np.ascontiguousarray(Wq[rows, :].T).astype(BF16),
            "wkt": np.ascontiguousarray(Wk[rows, :].T).astype(BF16),
            "wvt": np.ascontiguousarray(Wv[sl, :].T).astype(BF16),
            "wot": np.ascontiguousarray(Wo[:, sl].T).astype(BF16),
            "cost": cosP,
            "sint": sinP,
            "masks": masks_bf,
        })
    return in_maps


_CACHED_NC = None
_LAST_RESULTS = None


def kernel(hidden_states, cos, sin, Wq, Wk, Wv, Wo):
    global _CACHED_NC, _LAST_RESULTS
    in_maps = make_in_maps(hidden_states, cos, sin, Wq, Wk, Wv, Wo)
    if _CACHED_NC is None:
        _CACHED_NC = build_nc()
    res = run_bass_kernel_spmd(_CACHED_NC, in_maps, core_ids=list(range(NCORES)))
    _LAST_RESULTS = res
    acc = np.zeros((B * S, H), np.float32)
    for r in res.results:
        acc += r["out"].astype(np.float32)
    return acc.reshape(B, S, H)
